# revision 1
# baseline (speedup 1.0000x reference)
"""Attention-pooling kernel (AttLayer) for Trainium2, data-parallel over batch
across 8 NeuronCores.

  uit = tanh(x @ W + b)            [B, T, A]
  ait = exp(uit @ u) * mask        [B, T]
  out = einsum('btd,bt->bd', x, ait / (sum_t ait + eps))

Shapes hardcoded: x [64, 4096, 256] f32, W [256, 32], b [32], u [32, 1],
mask [64, 4096] bool. Each core handles 8 batches.

Design (DMA-bound; ~135-138 us measured vs 94 us HBM floor, baseline 151-162):
- x loads are SWDGE cast-DMAs (f32 DRAM -> bf16 SBUF), one per supergroup
  tile [128, 16 chunks, 128 f32-pairs]. t = 2048 g + 16 p + r. Runs near
  HBM line rate; the cast is exact RNE.  Tiles are sized to the DMA
  granularity because dependency regions through bitcast views widen to
  the whole tile -- per-SG tiles make that widening equal the transfer
  itself, so transposes/phase-3 overlap the stream cleanly.
- Per chunk: ONE f32 [128,128] PE "pair" transpose moves the whole
  [128t, 256d] bf16 chunk (fp32r mantissa loss ~2^-16 of pair max; far
  below bf16 noise).  xt[P, 2s+j] = x[t_s, 2P+j].
- x@W uses parity-packed W as the *stationary* operand: per quad q (4
  chunks), matmuls write uit^T [32 a, 512 (c s)] at PSUM partitions 32q via
  tile_position=(0, 32q); q is the inner loop so 4 col-groups stream
  CONCURRENTLY (~3x).  PSUM "start" clears has_written for the written
  partitions across the full bank width -> start=True per quad.
- One [128, 512] tanh(+per-partition bias) per supergroup on all 128
  partitions; scores via matmul(lhsT=tanh slice, rhs=block-diagonal u4)
  put t back on partitions -- exactly phase-3's lhsT layout.
- Phase 3 (weighted sum) is also 4-way col-tiled into one [128, 512] PSUM
  tile; a [128, 2]-selector matmul folds the 4 regions into the [2, 512]
  diagonal-block layout; exp row-sums feed a [128,1]x[128,1] denominator
  matmul.  Mask enters as an additive pre-exp bias.
- Tails (mask/exp/den/phase-3) are emitted per supergroup, and pool tiles
  are allocated lazily at first write so bufs=1 PSUM pools never see
  late references (pool-rotation WAR races).
"""

import os
import sys

sys.path.insert(0, "/opt/trn_rl_repo")

import numpy as np

import concourse.bass as bass
import concourse.mybir as mybir
import concourse.tile as tile
from concourse import bacc
from concourse.bass import ds, ts
from concourse import bass_utils
from concourse.bass_utils import run_bass_kernel_spmd

F32 = mybir.dt.float32
BF16 = mybir.dt.bfloat16

N_CORES = 8
B, T, D, A = 64, 4096, 256, 32
BPC = B // N_CORES          # batches per core
NCH = T // 128              # 128-row chunks per batch (32)
NG = 2                      # supergroups (DMA slabs) per batch
RPG = NCH // NG             # chunks per supergroup (16)
NQ = 4                      # quads per supergroup
EPS = 1e-7
MASK_BIAS = 30.0            # additive pre-exp mask: s + (mask-1)*30

last_exec_time_ns = None
last_result = None


DEBUG = bool(int(os.environ.get("BASS_V2_DEBUG", "0")))


def _build():
    nc = bacc.Bacc(None, target_bir_lowering=False, debug=True)

    x_dram = nc.dram_tensor("x", [BPC, T, D], F32, kind="ExternalInput")
    w2_dram = nc.dram_tensor("w2", [128, 2 * A], F32, kind="ExternalInput")
    u4_dram = nc.dram_tensor("u4", [128, NQ], F32, kind="ExternalInput")
    sel2_dram = nc.dram_tensor("sel2", [128, 2], F32, kind="ExternalInput")
    b4_dram = nc.dram_tensor("b4", [128, 1], F32, kind="ExternalInput")
    maskb_dram = nc.dram_tensor("maskb", [BPC, 128, NCH], F32, kind="ExternalInput")
    ident_dram = nc.dram_tensor("ident", [128, 128], F32, kind="ExternalInput")
    out_dram = nc.dram_tensor("out", [BPC, D], F32, kind="ExternalOutput")
    if DEBUG:
        dbg_xt = nc.dram_tensor("dbg_xt", [128, 512], F32, kind="ExternalOutput")
        dbg_uit = nc.dram_tensor("dbg_uit", [128, 512], F32, kind="ExternalOutput")
        dbg_tanh = nc.dram_tensor("dbg_tanh", [128, 512], F32, kind="ExternalOutput")
        dbg_sm = nc.dram_tensor("dbg_sm", [2, 128, NCH], F32, kind="ExternalOutput")
        dbg_ebf = nc.dram_tensor("dbg_ebf", [2, 128, NCH], F32, kind="ExternalOutput")
        dbg_uit5 = nc.dram_tensor("dbg_uit5", [128, 512], F32, kind="ExternalOutput")
        dbg_o2 = nc.dram_tensor("dbg_o2", [2, 2, 2 * D], F32, kind="ExternalOutput")
        dbg_den = nc.dram_tensor("dbg_den", [2, 1, 1], F32, kind="ExternalOutput")

    with tile.TileContext(nc) as tc:
        with (
            tc.tile_pool(name="const", bufs=1) as cpool,
            tc.tile_pool(name="xb", bufs=6) as xbpool,
            tc.tile_pool(name="xt", bufs=5) as xtpool,
            tc.tile_pool(name="th", bufs=3) as thpool,
            tc.tile_pool(name="small", bufs=2) as spool,
            tc.tile_pool(name="xtps", bufs=2, space="PSUM") as xtpspool,
            tc.tile_pool(name="uitps", bufs=2, space="PSUM") as uitpool,
            tc.tile_pool(name="eps", bufs=1, space="PSUM") as epool,
            tc.tile_pool(name="ops", bufs=1, space="PSUM") as opool,
            tc.tile_pool(name="o2ps", bufs=1, space="PSUM") as o2pool,
            tc.tile_pool(name="denps", bufs=1, space="PSUM") as denpool,
        ):
            # ---- constants (one-time) ----
            w2_f = cpool.tile([128, 2 * A], F32, name="w2_f")
            nc.sync.dma_start(out=w2_f[:], in_=w2_dram[:])
            w2_bf = cpool.tile([128, 2 * A], BF16, name="w2_bf")
            nc.vector.tensor_copy(w2_bf[:], w2_f[:])

            u4_f = cpool.tile([128, NQ], F32, name="u4_f")
            nc.sync.dma_start(out=u4_f[:], in_=u4_dram[:])
            u4_bf = cpool.tile([128, NQ], BF16, name="u4_bf")
            nc.vector.tensor_copy(u4_bf[:], u4_f[:])

            sel2 = cpool.tile([128, 2], F32, name="sel2")
            nc.sync.dma_start(out=sel2[:], in_=sel2_dram[:])

            b4 = cpool.tile([128, 1], F32, name="b4")
            nc.sync.dma_start(out=b4[:], in_=b4_dram[:])

            ident = cpool.tile([128, 128], F32, name="ident")
            nc.sync.dma_start(out=ident[:], in_=ident_dram[:])

            ones_f = cpool.tile([128, 1], F32, name="ones_f")
            nc.vector.memset(ones_f[:], 1.0)

            # deferred-emission state for software pipelining
            pend_score = [None]   # (tanh_sb, bb, g)
            pend_tail = [None]    # (bb, g) whose mask/exp/den/ph3 is pending
            e_tiles = {}          # bb -> e_ps tile, allocated at first write
            den_tiles = {}        # bb -> den psum tile
            o_tiles = {}          # bb -> phase-3 psum tile
            mb_tiles = {}         # bb -> maskb tile
    
            def emit_score(work):
                tanh_sb, bb, g = work
                # allocate e_ps(bb) lazily HERE so every reference to the
                # previous batch's e_ps tile is already emitted when the pool
                # rotates (bufs=1) -- else the tail's read races next scores.
                if bb not in e_tiles:
                    e_tiles[bb] = epool.tile([128, NCH], F32, name="e_ps", tag="e")
                e_v = e_tiles[bb].rearrange("p (g q c) -> p g q c", g=NG, q=NQ)
                for c in range(4):
                    nc.tensor.matmul(
                        e_v[:, g, :, c],
                        lhsT=tanh_sb[:, ds(128 * c, 128)],
                        rhs=u4_bf[:],
                        start=True,
                        stop=True,
                    )
                pend_tail[0] = (bb, g)

            def emit_sg_tail(bb, g):
                """Mask+exp+den+phase-3 for the 16 chunks of SG (bb, g)."""
                e_ps = e_tiles[bb]
                x_bf = xb_tiles[(bb, g)]
                s_m = spool.tile([128, RPG], F32, name="s_m", tag="s_m")
                nc.vector.tensor_add(
                    s_m[:], e_ps[:, ds(RPG * g, RPG)],
                    mb_tiles[bb][:, ds(RPG * g, RPG)],
                )
                e_bf = spool.tile([128, RPG], BF16, name="e_bf", tag="e_bf")
                er = spool.tile([128, 1], F32, name="er", tag="er")
                nc.scalar.activation(
                    e_bf[:], s_m[:], mybir.ActivationFunctionType.Exp,
                    accum_out=er[:],
                )
                if g == 0:
                    den_tiles[bb] = denpool.tile([1, 1], F32, name="den_ps", tag="den")
                    o_tiles[bb] = opool.tile([128, 2 * D], F32, name="o_ps", tag="o")
                nc.tensor.matmul(
                    den_tiles[bb][:], lhsT=er[:], rhs=ones_f[:],
                    start=(g == 0), stop=(g == 1),
                )
                o_ps = o_tiles[bb]
                for k in range(RPG // 2):
                    cg = k % 4
                    nc.tensor.matmul(
                        o_ps[ds(32 * cg, 2), :],
                        lhsT=e_bf[:, ds(2 * k, 2)],
                        rhs=x_bf[:, ds(2 * k, 2), :],
                        start=(g == 0 and k < 4),
                        stop=(g == 1 and k >= 4),
                        tile_position=(0, 32 * cg),
                    )

            def emit_batch_final(bb):
                e_tiles.pop(bb)
                mb_tiles.pop(bb)
                xb_tiles.pop((bb, 0))
                xb_tiles.pop((bb, 1))
                o_ps = o_tiles.pop(bb)
                den_ps = den_tiles.pop(bb)
                o2f = spool.tile([128, 2 * D], F32, name="o2f", tag="o2f")
                nc.vector.tensor_copy(o2f[:], o_ps[:])
                o2_ps = o2pool.tile([2, 2 * D], F32, name="o2_ps", tag="o2")
                nc.tensor.matmul(
                    o2_ps[:], lhsT=sel2[:], rhs=o2f[:], start=True, stop=True
                )
                den_sb = spool.tile([1, 1], F32, name="den_sb", tag="den_sb")
                nc.vector.tensor_scalar_add(den_sb[:], den_ps[:], EPS)
                inv = spool.tile([1, 1], F32, name="inv", tag="inv")
                nc.vector.reciprocal(inv[:], den_sb[:])
                o2_sb = spool.tile([2, 2 * D], F32, name="o2_sb", tag="o2_sb")
                nc.vector.tensor_copy(o2_sb[:], o2_ps[:])
                o_hi = spool.tile([1, D], F32, name="o_hi", tag="o_hi")
                nc.sync.dma_start(out=o_hi[:], in_=o2_sb[1:2, ds(D, D)])
                o_sum = spool.tile([1, D], F32, name="o_sum", tag="o_sum")
                nc.vector.tensor_add(o_sum[:], o2_sb[0:1, 0:D], o_hi[:])
                o_sb = spool.tile([1, D], F32, name="o_sb", tag="o_sb")
                nc.vector.tensor_scalar_mul(o_sb[:], o_sum[:], inv[:])
                nc.sync.dma_start(out=out_dram[bb][None, :], in_=o_sb[:])

            def emit_sg_dma(bb, g):
                """One cast-DMA per supergroup tile [128, 16, 128-pair] f32.
                At SG-tile granularity, whole-tile dependency widening from
                the bitcast write equals the DMA's own region, so readers
                (transposes, phase 3) overlap cleanly with the stream."""
                # BF16 tile + PLAIN dma writes: a bitcast on the DMA out AP
                # widens the dependency beyond the tile (pool-wide), making
                # every reader wait on the newest issued x DMA.  Only the
                # transposes bitcast-read, and at per-SG granularity even a
                # widened read equals exactly this tile's own DMA.
                x_bf = xbpool.tile([128, RPG, D], BF16, name="x_bf", tag="xb")
                x_view = x_dram[bb][ds(2048 * g, 2048), :].rearrange(
                    "(p r) d -> p r d", r=RPG
                )
                if (bb == 0 and g == 0) or (bb == BPC - 1 and g == 1):
                    # split the first SG (pipeline fill) and the last SG
                    # (tail overlap) so compute starts on partial data
                    for k in range(2):
                        nc.gpsimd.dma_start(
                            out=x_bf[:, ds(8 * k, 8), :],
                            in_=x_view[:, ds(8 * k, 8), :],
                        )
                else:
                    nc.gpsimd.dma_start(out=x_bf[:], in_=x_view)
                if g == 0:
                    maskb = spool.tile([128, NCH], F32, name="maskb", tag="maskb")
                    nc.sync.dma_start(out=maskb[:], in_=maskb_dram[bb])
                    mb_tiles[bb] = maskb
                xb_tiles[(bb, g)] = x_bf
                return x_bf

            def emit_sg(bb, g, x_bf):
                """One supergroup: transposes, x@W, tanh."""
                uit_ps = uitpool.tile([128, 4 * 128], F32, name="uit_ps", tag="uit")
                # pair-trick transposes: one f32 [128,128] PE transpose moves a
                # whole [128t, 256d] bf16 chunk (adjacent bf16 pairs ride as one
                # f32; fp32r mantissa loss is ~2^-16 of the pair max -- far
                # below bf16 noise).  xt[P, 2s+j] = x[t_s, 2P+j].
                xt_sbs = []
                for q in range(NQ):
                    xt_ps = xtpspool.tile([128, 4, 128], F32, name="xt_ps", tag="xtps")
                    for cc in range(4):
                        nc.tensor.transpose(
                            xt_ps[:, cc, :],
                            x_bf[:, 4 * q + cc, :].bitcast(F32),
                            ident[:],
                        )
                    xt_sb = xtpool.tile([128, 4, 128], F32, name="xt_sb", tag="xt")
                    if q % 2 == 0:
                        nc.vector.tensor_copy(xt_sb[:], xt_ps[:])
                    else:
                        nc.scalar.copy(xt_sb[:], xt_ps[:])
                    xt_sbs.append(xt_sb[:].bitcast(BF16))  # [128, 4, 256]

                # deferred from previous SG: its score matmuls (tanh is ready)
                if pend_score[0] is not None:
                    emit_score(pend_score[0])

                # x@W: quad q accumulates uit^T into PSUM partitions 32q via
                # tile_position col-groups; q is the inner loop so the four
                # col-groups stream concurrently.  The start clear resets
                # has_written for the WRITTEN PARTITIONS across the full bank
                # width, so each quad's first matmul must set start=True.
                for j in range(2):
                    for cc in range(4):
                        for q in range(NQ):
                            nc.tensor.matmul(
                                uit_ps[ds(32 * q, 32), ds(128 * cc, 128)],
                                lhsT=w2_bf[:, ds(A * j, A)],
                                rhs=xt_sbs[q][:, cc, ds(j, 128, step=2)],
                                start=(j == 0 and cc == 0),
                                stop=(j == 1 and cc == 3 and q == NQ - 1),
                                tile_position=(0, 32 * q),
                            )

                if pend_tail[0] is not None:
                    tb, tg = pend_tail[0]
                    pend_tail[0] = None
                    emit_sg_tail(tb, tg)
                    if tg == 1:
                        emit_batch_final(tb)

                tanh_sb = thpool.tile([128, 4 * 128], BF16, name="tanh_sb", tag="th")
                nc.scalar.activation(
                    tanh_sb[:],
                    uit_ps[:],
                    mybir.ActivationFunctionType.Tanh,
                    bias=b4[:],
                )
                if DEBUG and bb == 0 and g == 0:
                    t1 = spool.tile([128, 512], F32, name="dbg1", tag="dbg1")
                    nc.vector.tensor_copy(
                        t1[:], xt_sbs[0].bitcast(F32).rearrange("p c s -> p (c s)")
                    )
                    nc.sync.dma_start(out=dbg_xt[:], in_=t1[:])
                    t2 = spool.tile([128, 512], F32, name="dbg2", tag="dbg2")
                    nc.vector.tensor_copy(t2[:], uit_ps[:])
                    nc.sync.dma_start(out=dbg_uit[:], in_=t2[:])
                    t3 = spool.tile([128, 512], F32, name="dbg3", tag="dbg3")
                    nc.vector.tensor_copy(t3[:], tanh_sb[:])
                    nc.sync.dma_start(out=dbg_tanh[:], in_=t3[:])
                if DEBUG and bb == 5 and g == 0:
                    t2b = spool.tile([128, 512], F32, name="dbg2b", tag="dbg2b")
                    nc.vector.tensor_copy(t2b[:], uit_ps[:])
                    nc.sync.dma_start(out=dbg_uit5[:], in_=t2b[:])
                pend_score[0] = (tanh_sb, bb, g)
                return x_bf

            # pipeline: the pend_score hook (fired inside the next SG's
            # emission) emits scores, then that SG's mask/exp/den/phase-3,
            # and on g==1 the batch finalization.
            xb_tiles = {}
            for bb in range(BPC):
                x0 = emit_sg_dma(bb, 0)
                emit_sg(bb, 0, x0)
                x1 = emit_sg_dma(bb, 1)
                emit_sg(bb, 1, x1)
            emit_score(pend_score[0])
            tb, tg = pend_tail[0]
            emit_sg_tail(tb, tg)
            if tg == 1:
                emit_batch_final(tb)

    nc.finalize()
    return nc


def kernel(x, mask, W, b, u):
    global last_exec_time_ns, last_result
    x = np.ascontiguousarray(np.asarray(x), dtype=np.float32)
    mask_f = np.asarray(mask).astype(np.float32)
    W = np.asarray(W, dtype=np.float32)
    b = np.asarray(b, dtype=np.float32)
    u = np.asarray(u, dtype=np.float32)

    # host-side layout prep (all tiny; x is only view-sliced)
    # d-parity packing to match pair-trick transposes:
    # w2[p, A*j + a] = W[2p + j, a]
    w2 = np.ascontiguousarray(W.reshape(128, 2 * A))
    # block-diagonal u: u4[32q + a, q] = u[a]
    u4 = np.zeros((128, NQ), dtype=np.float32)
    for q in range(NQ):
        u4[32 * q : 32 * q + 32, q] = u[:, 0]
    # bias replicated per quad row-block
    b4 = np.ascontiguousarray(np.tile(b, NQ)[:, None])
    # mask -> additive pre-exp bias, laid out [b][p][(g r)] with t = 2048g+16p+r
    maskb = np.ascontiguousarray(
        ((mask_f - 1.0) * MASK_BIAS)
        .reshape(B, NG, 128, RPG)
        .transpose(0, 2, 1, 3)
        .reshape(B, 128, NCH)
    )
    ident = np.eye(128, dtype=np.float32)
    sel2 = np.zeros((128, 2), dtype=np.float32)
    for jj in range(4):
        sel2[32 * jj, 0] = 1.0
        sel2[32 * jj + 1, 1] = 1.0

    nc = _build()

    in_maps = []
    for c in range(N_CORES):
        in_maps.append(
            {
                "x": x[c * BPC : (c + 1) * BPC],
                "w2": w2,
                "u4": u4,
                "b4": b4,
                "maskb": maskb[c * BPC : (c + 1) * BPC],
                "ident": ident,
                "sel2": sel2,
            }
        )

    trace = bool(int(os.environ.get("BASS_KERNEL_TRACE", "0")))
    res = run_bass_kernel_spmd(
        nc, in_maps, core_ids=list(range(N_CORES)), trace=trace
    )
    last_exec_time_ns = res.exec_time_ns
    last_result = res

    out = np.empty((B, D), dtype=np.float32)
    for c in range(N_CORES):
        out[c * BPC : (c + 1) * BPC] = res.results[c]["out"]
    return out



# revision 10
# speedup vs baseline: 1.0712x; 1.0712x over previous
"""Attention-pooling kernel (AttLayer) for Trainium2, data-parallel over batch
across 8 NeuronCores.

  uit = tanh(x @ W + b)            [B, T, A]
  ait = exp(uit @ u) * mask        [B, T]
  out = einsum('btd,bt->bd', x, ait / (sum_t ait + eps))

Shapes hardcoded: x [64, 4096, 256] f32, W [256, 32], b [32], u [32, 1],
mask [64, 4096] bool. Each core handles 8 batches.

Design (DMA floor ~93 us; baseline v1 was 135-142 us):
- x loads are SWDGE cast-DMAs (f32 DRAM -> bf16 SBUF), QUAD-granular: one
  dma_start per 4 chunks [128, 4, 256].  t = 2048 g + 16 p + r.  Readers
  (bf16 transposes, phase-3) depend only on their own quad's DMA, so the
  PE starts ~10 us in and tracks the stream closely.
- Transposes are native bf16 [128,128] PE transposes (2 per chunk) into a
  bf16 PSUM tile (1 bank per quad).  bf16 streams at 1 row/cyc vs f32's
  2, and there is no bitcast so dependency regions stay exact.
- xt PSUM->SBUF eviction is split per quad: DVE copies chunks 0-2, ACT
  copies chunk 3, in parallel, so the drain keeps pace with the PE fill.
- x@W uses W halves [128, 32] as the stationary; per quad q the matmuls
  write uit^T [32 a, 512 (c s)] at PSUM partitions 32q via
  tile_position=(0, 32q); q is the inner loop so 4 col-groups stream
  concurrently.  PSUM "start" clears has_written for the written
  partitions across the full bank width -> start=True per quad.
- One [128, 512] tanh(+per-partition bias) per supergroup; scores via
  matmul(lhsT=tanh slice, rhs=block-diagonal u4) put t back on
  partitions -- exactly phase-3's lhsT layout.
- Phase 3 (weighted sum) is one [1, 256] matmul per chunk, rotated over
  PSUM partitions {0,32,64,96} (tile_position col-groups) so 4 streams
  accumulate concurrently into one [128, 256] PSUM tile; a [128, 1]
  ones-selector matmul folds the 4 rows into [1, 256] -- no diagonal
  unscramble or partition-move DMA needed.  Exp row-sums feed a
  [128,1]x[128,1] denominator matmul that lands on PARTITION 32 of the
  same o2 PSUM bank (the fold writes partition 0; start=True only clears
  has_written for written partitions, and matmul bases must be 32-
  aligned).  This frees a PSUM bank so the per-batch e-score tile is
  double-buffered (epool bufs=2), removing the inter-batch PE stall.
- Mask enters as an additive pre-exp bias.  Tails (mask/exp/den/phase-3)
  are emitted per supergroup, one SG late (software pipelining), and pool
  tiles are allocated lazily at first write so bufs=1 PSUM pools never
  see late references (pool-rotation WAR races).
"""

import os
import sys

sys.path.insert(0, "/opt/trn_rl_repo")

import numpy as np

import concourse.bass as bass
import concourse.mybir as mybir
import concourse.tile as tile
from concourse import bacc
from concourse.bass import ds, ts
from concourse import bass_utils
from concourse.bass_utils import run_bass_kernel_spmd

F32 = mybir.dt.float32
BF16 = mybir.dt.bfloat16

N_CORES = 8
B, T, D, A = 64, 4096, 256, 32
BPC = B // N_CORES          # batches per core
NCH = T // 128              # 128-row chunks per batch (32)
NG = 2                      # supergroups (DMA slabs) per batch
RPG = NCH // NG             # chunks per supergroup (16)
NQ = 4                      # quads per supergroup
EPS = 1e-7
MASK_BIAS = 30.0            # additive pre-exp mask: s + (mask-1)*30

last_exec_time_ns = None
last_result = None


def _build():
    nc = bacc.Bacc(None, target_bir_lowering=False, debug=True)

    x_dram = nc.dram_tensor("x", [BPC, T, D], F32, kind="ExternalInput")
    w2_dram = nc.dram_tensor("w2", [128, 2 * A], F32, kind="ExternalInput")
    u4_dram = nc.dram_tensor("u4", [128, NQ], F32, kind="ExternalInput")
    sel1_dram = nc.dram_tensor("sel1", [128, 1], F32, kind="ExternalInput")
    b4_dram = nc.dram_tensor("b4", [128, 1], F32, kind="ExternalInput")
    maskb_dram = nc.dram_tensor("maskb", [BPC, 128, NCH], F32, kind="ExternalInput")
    ident_dram = nc.dram_tensor("ident", [128, 128], F32, kind="ExternalInput")
    out_dram = nc.dram_tensor("out", [BPC, D], F32, kind="ExternalOutput")

    with tile.TileContext(nc) as tc:
        with (
            tc.tile_pool(name="const", bufs=1) as cpool,
            tc.tile_pool(name="xb", bufs=6) as xbpool,
            tc.tile_pool(name="xt", bufs=5) as xtpool,
            tc.tile_pool(name="th", bufs=3) as thpool,
            tc.tile_pool(name="small", bufs=2) as spool,
            tc.tile_pool(name="xtps", bufs=2, space="PSUM") as xtpspool,
            tc.tile_pool(name="uitps", bufs=2, space="PSUM") as uitpool,
            tc.tile_pool(name="eps", bufs=2, space="PSUM") as epool,
            tc.tile_pool(name="ops", bufs=1, space="PSUM") as opool,
            tc.tile_pool(name="o2ps", bufs=1, space="PSUM") as o2pool,
        ):
            # ---- constants (one-time) ----
            w2_f = cpool.tile([128, 2 * A], F32, name="w2_f")
            nc.sync.dma_start(out=w2_f[:], in_=w2_dram[:])
            w2_bf = cpool.tile([128, 2 * A], BF16, name="w2_bf")
            nc.vector.tensor_copy(w2_bf[:], w2_f[:])

            u4_f = cpool.tile([128, NQ], F32, name="u4_f")
            nc.sync.dma_start(out=u4_f[:], in_=u4_dram[:])
            u4_bf = cpool.tile([128, NQ], BF16, name="u4_bf")
            nc.vector.tensor_copy(u4_bf[:], u4_f[:])

            sel1 = cpool.tile([128, 1], F32, name="sel1")
            nc.sync.dma_start(out=sel1[:], in_=sel1_dram[:])

            b4 = cpool.tile([128, 1], F32, name="b4")
            nc.sync.dma_start(out=b4[:], in_=b4_dram[:])

            ident_f = cpool.tile([128, 128], F32, name="ident_f")
            nc.sync.dma_start(out=ident_f[:], in_=ident_dram[:])
            ident = cpool.tile([128, 128], BF16, name="ident")
            nc.vector.tensor_copy(ident[:], ident_f[:])

            ones_f = cpool.tile([128, 1], F32, name="ones_f")
            nc.vector.memset(ones_f[:], 1.0)

            # deferred-emission state for software pipelining
            pend_score = [None]   # (tanh_sb, bb, g)
            pend_tail = [None]    # (bb, g) whose mask/exp/den/ph3 is pending
            e_tiles = {}          # bb -> e_ps tile, allocated at first write
            o_tiles = {}          # bb -> phase-3 psum tile
            o2_tiles = {}         # bb -> o2 psum tile (fold rows 0-1, den row 2)
            mb_tiles = {}         # bb -> maskb tile

            def emit_score(work):
                tanh_sb, bb, g = work
                # allocate e_ps(bb) lazily HERE so every reference to the
                # 2-batches-ago e_ps tile is already emitted when the pool
                # rotates (bufs=2).
                if bb not in e_tiles:
                    e_tiles[bb] = epool.tile([128, NCH], F32, name="e_ps", tag="e")
                e_v = e_tiles[bb].rearrange("p (g q c) -> p g q c", g=NG, q=NQ)
                for c in range(4):
                    nc.tensor.matmul(
                        e_v[:, g, :, c],
                        lhsT=tanh_sb[:, ds(128 * c, 128)],
                        rhs=u4_bf[:],
                        start=True,
                        stop=True,
                    )
                pend_tail[0] = (bb, g)

            def emit_sg_tail(bb, g):
                """Mask+exp+den+phase-3 for the 16 chunks of SG (bb, g)."""
                e_ps = e_tiles[bb]
                x_bf = xb_tiles[(bb, g)]
                s_m = spool.tile([128, RPG], F32, name="s_m", tag="s_m")
                nc.vector.tensor_add(
                    s_m[:], e_ps[:, ds(RPG * g, RPG)],
                    mb_tiles[bb][:, ds(RPG * g, RPG)],
                )
                e_bf = spool.tile([128, RPG], BF16, name="e_bf", tag="e_bf")
                er = spool.tile([128, 1], F32, name="er", tag="er")
                nc.scalar.activation(
                    e_bf[:], s_m[:], mybir.ActivationFunctionType.Exp,
                    accum_out=er[:],
                )
                if g == 0:
                    o_tiles[bb] = opool.tile([128, D], F32, name="o_ps", tag="o")
                    o2_tiles[bb] = o2pool.tile([33, D], F32, name="o2_ps", tag="o2")
                # denominator accumulates on partition 32 of the o2 bank; the
                # fold matmul later writes partition 0 only, so its start=True
                # clear cannot touch this cell (and matmul output bases must
                # be 32-aligned).
                nc.tensor.matmul(
                    o2_tiles[bb][32:33, 0:1], lhsT=er[:], rhs=ones_f[:],
                    start=(g == 0), stop=(g == 1),
                    tile_position=(0, 32),
                )
                o_ps = o_tiles[bb]
                for k in range(RPG):
                    cg = k % 4
                    nc.tensor.matmul(
                        o_ps[ds(32 * cg, 1), :],
                        lhsT=e_bf[:, ds(k, 1)],
                        rhs=x_bf[:, k, :],
                        start=(g == 0 and k < 4),
                        stop=(g == 1 and k >= RPG - 4),
                        tile_position=(0, 32 * cg),
                    )

            def emit_batch_final(bb):
                e_tiles.pop(bb)
                mb_tiles.pop(bb)
                xb_tiles.pop((bb, 0))
                xb_tiles.pop((bb, 1))
                o_ps = o_tiles.pop(bb)
                o2_ps = o2_tiles.pop(bb)
                o2f = spool.tile([128, D], F32, name="o2f", tag="o2f")
                nc.vector.tensor_copy(o2f[:], o_ps[:])
                nc.tensor.matmul(
                    o2_ps[0:1, :], lhsT=sel1[:], rhs=o2f[:], start=True, stop=True
                )
                o2_sb = spool.tile([1, D], F32, name="o2_sb", tag="o2_sb")
                nc.vector.tensor_copy(o2_sb[:], o2_ps[0:1, :])
                den_sb = spool.tile([1, 1], F32, name="den_sb", tag="den_sb")
                nc.vector.tensor_scalar_add(den_sb[:], o2_ps[32:33, 0:1], EPS)
                inv = spool.tile([1, 1], F32, name="inv", tag="inv")
                nc.vector.reciprocal(inv[:], den_sb[:])
                o_sb = spool.tile([1, D], F32, name="o_sb", tag="o_sb")
                nc.vector.tensor_scalar_mul(o_sb[:], o2_sb[:], inv[:])
                nc.sync.dma_start(out=out_dram[bb][None, :], in_=o_sb[:])

            def emit_sg_dma(bb, g):
                """Quad-granular cast-DMAs: 4 per supergroup, each [128, 4
                chunks, 256] f32->bf16.  Readers touch x_bf natively (no
                bitcast), so each transpose / phase-3 matmul depends only on
                the one quad DMA that wrote its region."""
                x_bf = xbpool.tile([128, RPG, D], BF16, name="x_bf", tag="xb")
                x_view = x_dram[bb][ds(2048 * g, 2048), :].rearrange(
                    "(p r) d -> p r d", r=RPG
                )
                for qq in range(NQ):
                    nc.gpsimd.dma_start(
                        out=x_bf[:, ds(4 * qq, 4), :],
                        in_=x_view[:, ds(4 * qq, 4), :],
                    )
                if g == 0:
                    maskb = spool.tile([128, NCH], F32, name="maskb", tag="maskb")
                    nc.sync.dma_start(out=maskb[:], in_=maskb_dram[bb])
                    mb_tiles[bb] = maskb
                xb_tiles[(bb, g)] = x_bf
                return x_bf

            def emit_sg(bb, g, x_bf):
                """One supergroup: transposes, x@W, tanh."""
                uit_ps = uitpool.tile([128, 4 * 128], F32, name="uit_ps", tag="uit")
                # native bf16 transposes: per chunk, two [128,128] PE
                # transposes (d halves) into a bf16 PSUM tile.
                # xt[d', (c h t)] = x[t_c, 128 h + d'].
                xt_sbs = []
                for q in range(NQ):
                    xt_ps = xtpspool.tile([128, 4, 2, 128], BF16, name="xt_ps", tag="xtps")
                    for cc in range(4):
                        for h in range(2):
                            nc.tensor.transpose(
                                xt_ps[:, cc, h, :],
                                x_bf[:, 4 * q + cc, ds(128 * h, 128)],
                                ident[:],
                            )
                    xt_sb = xtpool.tile([128, 4, 2, 128], BF16, name="xt_sb", tag="xt")
                    # parallel drain: DVE takes chunks 0-2, ACT chunk 3
                    nc.vector.tensor_copy(xt_sb[:, 0:3], xt_ps[:, 0:3])
                    nc.scalar.copy(xt_sb[:, 3:4], xt_ps[:, 3:4])
                    xt_sbs.append(xt_sb)

                # deferred from previous SG: its score matmuls (tanh is ready)
                if pend_score[0] is not None:
                    emit_score(pend_score[0])

                # x@W: quad q accumulates uit^T into PSUM partitions 32q via
                # tile_position col-groups; q is the inner loop so the four
                # col-groups stream concurrently.  The start clear resets
                # has_written for the WRITTEN PARTITIONS across the full bank
                # width, so each quad's first matmul must set start=True.
                for h in range(2):
                    for cc in range(4):
                        for q in range(NQ):
                            nc.tensor.matmul(
                                uit_ps[ds(32 * q, 32), ds(128 * cc, 128)],
                                lhsT=w2_bf[:, ds(A * h, A)],
                                rhs=xt_sbs[q][:, cc, h, :],
                                start=(h == 0 and cc == 0),
                                stop=(h == 1 and cc == 3 and q == NQ - 1),
                                tile_position=(0, 32 * q),
                            )

                if pend_tail[0] is not None:
                    tb, tg = pend_tail[0]
                    pend_tail[0] = None
                    emit_sg_tail(tb, tg)
                    if tg == 1:
                        emit_batch_final(tb)

                tanh_sb = thpool.tile([128, 4 * 128], BF16, name="tanh_sb", tag="th")
                nc.scalar.activation(
                    tanh_sb[:],
                    uit_ps[:],
                    mybir.ActivationFunctionType.Tanh,
                    bias=b4[:],
                )
                pend_score[0] = (tanh_sb, bb, g)
                return x_bf

            # pipeline: the pend_score hook (fired inside the next SG's
            # emission) emits scores, then that SG's mask/exp/den/phase-3,
            # and on g==1 the batch finalization.
            xb_tiles = {}
            for bb in range(BPC):
                x0 = emit_sg_dma(bb, 0)
                emit_sg(bb, 0, x0)
                x1 = emit_sg_dma(bb, 1)
                emit_sg(bb, 1, x1)
            emit_score(pend_score[0])
            tb, tg = pend_tail[0]
            emit_sg_tail(tb, tg)
            if tg == 1:
                emit_batch_final(tb)

    nc.finalize()
    return nc


def kernel(x, mask, W, b, u):
    global last_exec_time_ns, last_result
    x = np.ascontiguousarray(np.asarray(x), dtype=np.float32)
    mask_f = np.asarray(mask).astype(np.float32)
    W = np.asarray(W, dtype=np.float32)
    b = np.asarray(b, dtype=np.float32)
    u = np.asarray(u, dtype=np.float32)

    # host-side layout prep (all tiny; x is only view-sliced)
    # d-half packing to match the native bf16 transposes:
    # w2[p, A*h + a] = W[128h + p, a]
    w2 = np.ascontiguousarray(
        W.reshape(2, 128, A).transpose(1, 0, 2).reshape(128, 2 * A)
    )
    # block-diagonal u: u4[32q + a, q] = u[a]
    u4 = np.zeros((128, NQ), dtype=np.float32)
    for q in range(NQ):
        u4[32 * q : 32 * q + 32, q] = u[:, 0]
    # bias replicated per quad row-block
    b4 = np.ascontiguousarray(np.tile(b, NQ)[:, None])
    # mask -> additive pre-exp bias, laid out [b][p][(g r)] with t = 2048g+16p+r
    maskb = np.ascontiguousarray(
        ((mask_f - 1.0) * MASK_BIAS)
        .reshape(B, NG, 128, RPG)
        .transpose(0, 2, 1, 3)
        .reshape(B, 128, NCH)
    )
    ident = np.eye(128, dtype=np.float32)
    sel1 = np.zeros((128, 1), dtype=np.float32)
    for jj in range(4):
        sel1[32 * jj, 0] = 1.0

    nc = _build()

    in_maps = []
    for c in range(N_CORES):
        in_maps.append(
            {
                "x": x[c * BPC : (c + 1) * BPC],
                "w2": w2,
                "u4": u4,
                "b4": b4,
                "maskb": maskb[c * BPC : (c + 1) * BPC],
                "ident": ident,
                "sel1": sel1,
            }
        )

    trace = bool(int(os.environ.get("BASS_KERNEL_TRACE", "0")))
    res = run_bass_kernel_spmd(
        nc, in_maps, core_ids=list(range(N_CORES)), trace=trace
    )
    last_exec_time_ns = res.exec_time_ns
    last_result = res

    out = np.empty((B, D), dtype=np.float32)
    for c in range(N_CORES):
        out[c * BPC : (c + 1) * BPC] = res.results[c]["out"]
    return out


# revision 14
# speedup vs baseline: 1.0873x; 1.0150x over previous
"""Attention-pooling kernel (AttLayer) for Trainium2, data-parallel over batch
across 8 NeuronCores.

  uit = tanh(x @ W + b)            [B, T, A]
  ait = exp(uit @ u) * mask        [B, T]
  out = einsum('btd,bt->bd', x, ait / (sum_t ait + eps))

Shapes hardcoded: x [64, 4096, 256] f32, W [256, 32], b [32], u [32, 1],
mask [64, 4096] bool. Each core handles 8 batches.

Design (DMA floor ~93 us; baseline v1 was 135-142 us):
- x loads are SWDGE cast-DMAs (f32 DRAM -> bf16 SBUF), QUAD-granular: one
  dma_start per 4 chunks [128, 4, 256].  t = 2048 g + 16 p + r.  Readers
  (bf16 transposes, phase-3) depend only on their own quad's DMA, so the
  PE starts ~10 us in and tracks the stream closely.
- Transposes are native bf16 [128,128] PE transposes (2 per chunk) into a
  bf16 PSUM tile (1 bank per quad).  bf16 streams at 1 row/cyc vs f32's
  2, and there is no bitcast so dependency regions stay exact.
- xt PSUM->SBUF eviction is split per quad: DVE copies chunks 0-2, ACT
  copies chunk 3, in parallel, so the drain keeps pace with the PE fill.
- x@W uses W halves [128, 32] as the stationary; per quad q the matmuls
  write uit^T [32 a, 512 (c s)] at PSUM partitions 32q via
  tile_position=(0, 32q); q is the inner loop so 4 col-groups stream
  concurrently.  PSUM "start" clears has_written for the written
  partitions across the full bank width -> start=True per quad.
- One [128, 512] tanh(+per-partition bias) per supergroup; scores via
  matmul(lhsT=tanh slice, rhs=block-diagonal u4) put t back on
  partitions -- exactly phase-3's lhsT layout.
- Phase 3 (weighted sum) is one [1, 256] matmul per chunk, rotated over
  PSUM partitions {0,32,64,96} (tile_position col-groups) so 4 streams
  accumulate concurrently into one [128, 256] PSUM tile; a [128, 1]
  ones-selector matmul folds the 4 rows into [1, 256] -- no diagonal
  unscramble or partition-move DMA needed.  Exp row-sums feed a
  [128,1]x[128,1] denominator matmul that lands on PARTITION 32 of the
  same o2 PSUM bank (the fold writes partition 0; start=True only clears
  has_written for written partitions, and matmul bases must be 32-
  aligned).  This frees a PSUM bank so the per-batch e-score tile is
  double-buffered (epool bufs=2), removing the inter-batch PE stall.
- Mask enters as an additive pre-exp bias.  Tails (mask/exp/den/phase-3)
  are emitted per supergroup, one SG late (software pipelining), and pool
  tiles are allocated lazily at first write so bufs=1 PSUM pools never
  see late references (pool-rotation WAR races).
"""

import os
import sys

sys.path.insert(0, "/opt/trn_rl_repo")

import numpy as np

import concourse.bass as bass
import concourse.mybir as mybir
import concourse.tile as tile
from concourse import bacc
from concourse.bass import ds, ts
from concourse import bass_utils
from concourse.bass_utils import run_bass_kernel_spmd

F32 = mybir.dt.float32
BF16 = mybir.dt.bfloat16

N_CORES = 8
B, T, D, A = 64, 4096, 256, 32
BPC = B // N_CORES          # batches per core
NCH = T // 128              # 128-row chunks per batch (32)
NG = 2                      # supergroups (DMA slabs) per batch
RPG = NCH // NG             # chunks per supergroup (16)
NQ = 4                      # quads per supergroup
EPS = 1e-7
MASK_BIAS = 30.0            # additive pre-exp mask: s + (mask-1)*30

last_exec_time_ns = None
last_result = None


def _build():
    nc = bacc.Bacc(None, target_bir_lowering=False, debug=True)

    x_dram = nc.dram_tensor("x", [BPC, T, D], F32, kind="ExternalInput")
    w2_dram = nc.dram_tensor("w2", [128, 2 * A], F32, kind="ExternalInput")
    u4_dram = nc.dram_tensor("u4", [128, NQ], F32, kind="ExternalInput")
    sel1_dram = nc.dram_tensor("sel1", [128, 1], F32, kind="ExternalInput")
    b4_dram = nc.dram_tensor("b4", [128, 1], F32, kind="ExternalInput")
    maskb_dram = nc.dram_tensor("maskb", [BPC, 128, NCH], F32, kind="ExternalInput")
    ident_dram = nc.dram_tensor("ident", [128, 128], F32, kind="ExternalInput")
    out_dram = nc.dram_tensor("out", [BPC, D], F32, kind="ExternalOutput")

    with tile.TileContext(nc) as tc:
        with (
            tc.tile_pool(name="const", bufs=1) as cpool,
            tc.tile_pool(name="xb", bufs=6) as xbpool,
            tc.tile_pool(name="xt", bufs=5) as xtpool,
            tc.tile_pool(name="th", bufs=3) as thpool,
            tc.tile_pool(name="small", bufs=2) as spool,
            tc.tile_pool(name="xtps", bufs=2, space="PSUM") as xtpspool,
            tc.tile_pool(name="uitps", bufs=2, space="PSUM") as uitpool,
            tc.tile_pool(name="eps", bufs=2, space="PSUM") as epool,
            tc.tile_pool(name="ops", bufs=1, space="PSUM") as opool,
            tc.tile_pool(name="o2ps", bufs=1, space="PSUM") as o2pool,
        ):
            # ---- constants (one-time) ----
            w2_f = cpool.tile([128, 2 * A], F32, name="w2_f")
            nc.sync.dma_start(out=w2_f[:], in_=w2_dram[:])
            w2_bf = cpool.tile([128, 2 * A], BF16, name="w2_bf")
            nc.vector.tensor_copy(w2_bf[:], w2_f[:])

            u4_f = cpool.tile([128, NQ], F32, name="u4_f")
            nc.sync.dma_start(out=u4_f[:], in_=u4_dram[:])
            u4_bf = cpool.tile([128, NQ], BF16, name="u4_bf")
            nc.vector.tensor_copy(u4_bf[:], u4_f[:])

            sel1 = cpool.tile([128, 1], F32, name="sel1")
            nc.sync.dma_start(out=sel1[:], in_=sel1_dram[:])

            b4 = cpool.tile([128, 1], F32, name="b4")
            nc.sync.dma_start(out=b4[:], in_=b4_dram[:])

            ident_f = cpool.tile([128, 128], F32, name="ident_f")
            nc.sync.dma_start(out=ident_f[:], in_=ident_dram[:])
            ident = cpool.tile([128, 128], BF16, name="ident")
            nc.vector.tensor_copy(ident[:], ident_f[:])

            ones_f = cpool.tile([128, 1], F32, name="ones_f")
            nc.vector.memset(ones_f[:], 1.0)

            # deferred-emission state for software pipelining
            pend_score = [None]   # (tanh_sb, bb, g)
            pend_tail = [None]    # (bb, g) whose mask/exp/den/ph3 is pending
            e_tiles = {}          # bb -> e_ps tile, allocated at first write
            o_tiles = {}          # bb -> phase-3 psum tile
            o2_tiles = {}         # bb -> o2 psum tile (fold rows 0-1, den row 2)
            mb_tiles = {}         # bb -> maskb tile

            def emit_score(work):
                tanh_sb, bb, g = work
                # allocate e_ps(bb) lazily HERE so every reference to the
                # 2-batches-ago e_ps tile is already emitted when the pool
                # rotates (bufs=2).
                if bb not in e_tiles:
                    e_tiles[bb] = epool.tile([128, NCH], F32, name="e_ps", tag="e")
                e_v = e_tiles[bb].rearrange("p (g q c) -> p g q c", g=NG, q=NQ)
                for c in range(4):
                    nc.tensor.matmul(
                        e_v[:, g, :, c],
                        lhsT=tanh_sb[:, ds(128 * c, 128)],
                        rhs=u4_bf[:],
                        start=True,
                        stop=True,
                    )
                pend_tail[0] = (bb, g)

            def emit_tail_exp(bb, g):
                """Mask+exp for SG (bb, g) -- the DVE/ACT part of the tail,
                emitted early so it runs before this SG's xt copies on those
                engines and phase-3's inputs are ready without a PE stall."""
                e_ps = e_tiles[bb]
                s_m = spool.tile([128, RPG], F32, name="s_m", tag="s_m")
                nc.vector.tensor_add(
                    s_m[:], e_ps[:, ds(RPG * g, RPG)],
                    mb_tiles[bb][:, ds(RPG * g, RPG)],
                )
                e_bf = spool.tile([128, RPG], BF16, name="e_bf", tag="e_bf")
                er = spool.tile([128, 1], F32, name="er", tag="er")
                nc.scalar.activation(
                    e_bf[:], s_m[:], mybir.ActivationFunctionType.Exp,
                    accum_out=er[:],
                )
                return e_bf, er

            def emit_tail_mm(bb, g, e_bf, er):
                """Denominator + phase-3 matmuls for SG (bb, g)."""
                x_bf = xb_tiles[(bb, g)]
                if g == 0:
                    o_tiles[bb] = opool.tile([128, D], F32, name="o_ps", tag="o")
                    o2_tiles[bb] = o2pool.tile([33, D], F32, name="o2_ps", tag="o2")
                # denominator accumulates on partition 32 of the o2 bank; the
                # fold matmul later writes partition 0 only, so its start=True
                # clear cannot touch this cell (and matmul output bases must
                # be 32-aligned).
                nc.tensor.matmul(
                    o2_tiles[bb][32:33, 0:1], lhsT=er[:], rhs=ones_f[:],
                    start=(g == 0), stop=(g == 1),
                    tile_position=(0, 32),
                )
                o_ps = o_tiles[bb]
                for k in range(RPG):
                    cg = k % 4
                    nc.tensor.matmul(
                        o_ps[ds(32 * cg, 1), :],
                        lhsT=e_bf[:, ds(k, 1)],
                        rhs=x_bf[:, k, :],
                        start=(g == 0 and k < 4),
                        stop=(g == 1 and k >= RPG - 4),
                        tile_position=(0, 32 * cg),
                    )

            def emit_batch_final(bb):
                e_tiles.pop(bb)
                mb_tiles.pop(bb)
                xb_tiles.pop((bb, 0))
                xb_tiles.pop((bb, 1))
                o_ps = o_tiles.pop(bb)
                o2_ps = o2_tiles.pop(bb)
                o2f = spool.tile([128, D], F32, name="o2f", tag="o2f")
                nc.vector.tensor_copy(o2f[:], o_ps[:])
                nc.tensor.matmul(
                    o2_ps[0:1, :], lhsT=sel1[:], rhs=o2f[:], start=True, stop=True
                )
                o2_sb = spool.tile([1, D], F32, name="o2_sb", tag="o2_sb")
                nc.vector.tensor_copy(o2_sb[:], o2_ps[0:1, :])
                den_sb = spool.tile([1, 1], F32, name="den_sb", tag="den_sb")
                nc.vector.tensor_scalar_add(den_sb[:], o2_ps[32:33, 0:1], EPS)
                inv = spool.tile([1, 1], F32, name="inv", tag="inv")
                nc.vector.reciprocal(inv[:], den_sb[:])
                o_sb = spool.tile([1, D], F32, name="o_sb", tag="o_sb")
                nc.vector.tensor_scalar_mul(o_sb[:], o2_sb[:], inv[:])
                nc.sync.dma_start(out=out_dram[bb][None, :], in_=o_sb[:])

            def emit_sg_dma(bb, g):
                """Cast-DMA per supergroup [128, 16 chunks, 256] f32->bf16.
                One SWDGE trigger each (~128 descriptors, ~0.6us of gpsimd);
                batch 0 is quad-granular instead so the PE can start on the
                first 512KB.  Readers touch x_bf natively (no bitcast), so
                dependency regions stay exact per DMA instruction."""
                x_bf = xbpool.tile([128, RPG, D], BF16, name="x_bf", tag="xb")
                x_view = x_dram[bb][ds(2048 * g, 2048), :].rearrange(
                    "(p r) d -> p r d", r=RPG
                )
                if bb == 0:
                    for qq in range(NQ):
                        nc.gpsimd.dma_start(
                            out=x_bf[:, ds(4 * qq, 4), :],
                            in_=x_view[:, ds(4 * qq, 4), :],
                        )
                else:
                    nc.gpsimd.dma_start(out=x_bf[:], in_=x_view)
                if g == 0:
                    maskb = spool.tile([128, NCH], F32, name="maskb", tag="maskb")
                    nc.sync.dma_start(out=maskb[:], in_=maskb_dram[bb])
                    mb_tiles[bb] = maskb
                xb_tiles[(bb, g)] = x_bf
                return x_bf

            def emit_sg(bb, g, x_bf):
                """One supergroup.  PE order: scores(prev) | transposes |
                phase-3(prev) | x@W | [tanh].  The previous SG's exp chain
                (DVE/ACT) is emitted before this SG's xt copies so phase-3's
                inputs are ready, and phase-3 between transposes and x@W
                gives the copies time to land before x@W streams them."""
                # deferred from previous SG: its score matmuls (tanh is ready)
                tail_work = None
                if pend_score[0] is not None:
                    emit_score(pend_score[0])
                    tb, tg = pend_tail[0]
                    pend_tail[0] = None
                    tail_work = (tb, tg, *emit_tail_exp(tb, tg))

                uit_ps = uitpool.tile([128, 4 * 128], F32, name="uit_ps", tag="uit")
                # native bf16 transposes: per chunk, two [128,128] PE
                # transposes (d halves) into a bf16 PSUM tile.
                # xt[d', (c h t)] = x[t_c, 128 h + d'].
                xt_sbs = []
                for q in range(NQ):
                    xt_ps = xtpspool.tile([128, 4, 2, 128], BF16, name="xt_ps", tag="xtps")
                    for cc in range(4):
                        for h in range(2):
                            nc.tensor.transpose(
                                xt_ps[:, cc, h, :],
                                x_bf[:, 4 * q + cc, ds(128 * h, 128)],
                                ident[:],
                            )
                    xt_sb = xtpool.tile([128, 4, 2, 128], BF16, name="xt_sb", tag="xt")
                    # parallel drain: DVE takes chunks 0-2, ACT chunk 3
                    nc.vector.tensor_copy(xt_sb[:, 0:3], xt_ps[:, 0:3])
                    nc.scalar.copy(xt_sb[:, 3:4], xt_ps[:, 3:4])
                    xt_sbs.append(xt_sb)

                if tail_work is not None:
                    tb, tg, e_bf, er = tail_work
                    emit_tail_mm(tb, tg, e_bf, er)
                    if tg == 1:
                        emit_batch_final(tb)

                # x@W: quad q accumulates uit^T into PSUM partitions 32q via
                # tile_position col-groups; q is the inner loop so the four
                # col-groups stream concurrently.  The start clear resets
                # has_written for the WRITTEN PARTITIONS across the full bank
                # width, so each quad's first matmul must set start=True.
                for h in range(2):
                    for cc in range(4):
                        for q in range(NQ):
                            nc.tensor.matmul(
                                uit_ps[ds(32 * q, 32), ds(128 * cc, 128)],
                                lhsT=w2_bf[:, ds(A * h, A)],
                                rhs=xt_sbs[q][:, cc, h, :],
                                start=(h == 0 and cc == 0),
                                stop=(h == 1 and cc == 3 and q == NQ - 1),
                                tile_position=(0, 32 * q),
                            )

                tanh_sb = thpool.tile([128, 4 * 128], BF16, name="tanh_sb", tag="th")
                nc.scalar.activation(
                    tanh_sb[:],
                    uit_ps[:],
                    mybir.ActivationFunctionType.Tanh,
                    bias=b4[:],
                )
                pend_score[0] = (tanh_sb, bb, g)
                return x_bf

            # pipeline: the pend_score hook (fired inside the next SG's
            # emission) emits scores, then that SG's mask/exp/den/phase-3,
            # and on g==1 the batch finalization.
            xb_tiles = {}
            for bb in range(BPC):
                x0 = emit_sg_dma(bb, 0)
                emit_sg(bb, 0, x0)
                x1 = emit_sg_dma(bb, 1)
                emit_sg(bb, 1, x1)
            emit_score(pend_score[0])
            tb, tg = pend_tail[0]
            e_bf, er = emit_tail_exp(tb, tg)
            emit_tail_mm(tb, tg, e_bf, er)
            if tg == 1:
                emit_batch_final(tb)

    nc.finalize()
    return nc


def kernel(x, mask, W, b, u):
    global last_exec_time_ns, last_result
    x = np.ascontiguousarray(np.asarray(x), dtype=np.float32)
    mask_f = np.asarray(mask).astype(np.float32)
    W = np.asarray(W, dtype=np.float32)
    b = np.asarray(b, dtype=np.float32)
    u = np.asarray(u, dtype=np.float32)

    # host-side layout prep (all tiny; x is only view-sliced)
    # d-half packing to match the native bf16 transposes:
    # w2[p, A*h + a] = W[128h + p, a]
    w2 = np.ascontiguousarray(
        W.reshape(2, 128, A).transpose(1, 0, 2).reshape(128, 2 * A)
    )
    # block-diagonal u: u4[32q + a, q] = u[a]
    u4 = np.zeros((128, NQ), dtype=np.float32)
    for q in range(NQ):
        u4[32 * q : 32 * q + 32, q] = u[:, 0]
    # bias replicated per quad row-block
    b4 = np.ascontiguousarray(np.tile(b, NQ)[:, None])
    # mask -> additive pre-exp bias, laid out [b][p][(g r)] with t = 2048g+16p+r
    maskb = np.ascontiguousarray(
        ((mask_f - 1.0) * MASK_BIAS)
        .reshape(B, NG, 128, RPG)
        .transpose(0, 2, 1, 3)
        .reshape(B, 128, NCH)
    )
    ident = np.eye(128, dtype=np.float32)
    sel1 = np.zeros((128, 1), dtype=np.float32)
    for jj in range(4):
        sel1[32 * jj, 0] = 1.0

    nc = _build()

    in_maps = []
    for c in range(N_CORES):
        in_maps.append(
            {
                "x": x[c * BPC : (c + 1) * BPC],
                "w2": w2,
                "u4": u4,
                "b4": b4,
                "maskb": maskb[c * BPC : (c + 1) * BPC],
                "ident": ident,
                "sel1": sel1,
            }
        )

    trace = bool(int(os.environ.get("BASS_KERNEL_TRACE", "0")))
    res = run_bass_kernel_spmd(
        nc, in_maps, core_ids=list(range(N_CORES)), trace=trace
    )
    last_exec_time_ns = res.exec_time_ns
    last_result = res

    out = np.empty((B, D), dtype=np.float32)
    for c in range(N_CORES):
        out[c * BPC : (c + 1) * BPC] = res.results[c]["out"]
    return out
